# revision 10
# baseline (speedup 1.0000x reference)
"""Dirichlet energy loss (ball-query KNN graph) on 8 Trainium2 cores.

For each point i in a cloud of N=4096 points: find its (up to) K=32 nearest
neighbors within radius R=0.15, sum (f_i - f_j)^2 over them, then return
0.5 * mean over all points/batches.

Strategy (data-parallel over B=8, one cloud per NeuronCore):
  host:   two-level spatial sort per cloud: 6 x-bins (fixed rank widths,
          multiples of 128), y-sorted inside each bin. All in-radius
          neighbors of a 128-row tile (always inside one bin) then lie in a
          few per-(tile, bin) rank bands computed EXACTLY via searchsorted
          (unioned over the 8 clouds so one SPMD program serves all cores;
          supersets stay correct). Precompute matmul operands so the device
          computes u_ij = r^2 - d^2_ij with one tiny-K matmul + one ACT op.
  device: per row tile: PE matmul (K=4 contraction) over the band columns ->
          2p_i.p_j - |p_j|^2 in PSUM; ACT adds per-row bias (r^2 - |p_i|^2)
          writing u0 in a 16-way interleaved "grouped" layout; 16 per-group
          vector.max ops give 128 survivors containing the top-32 (group g
          holds every 16th candidate, so the top-32 spread ~Bin(32,1/16)
          per group); a short max/match_replace chain on the survivors
          yields the 32nd-largest u (= distance threshold, clamped at 0 ==
          radius by a Relu); one fused scalar_tensor_tensor computes
          sum_j (u0 >= t) * (f_i - f_j)^2 per row (G = (f_i-f_j)^2 from ACT
          Square with per-partition bias, same grouped layout).
  host:   sum the per-row partials from all cores, multiply by 0.5/(B*N).
"""

import numpy as np

R = 0.15
RSQ = R * R
RPAD = R + 1e-4  # host window slack for fp32 distance rounding
K = 32
B = 8
N = 4096
NTILES = N // 128
NG = 16  # interleaved candidate groups per row
NBINS = 6
BIN_COUNTS = (640, 768, 640, 768, 640, 640)  # sum 4096, multiples of 128
BIN_EDGES = tuple(int(x) for x in np.cumsum((0,) + BIN_COUNTS))
BIG_NEG = -3.0e38
PSUM_W = 2048

_kernel_cache = {}


def _build_bass(windows, rep=1, hint=False):
    """windows: per tile, tuple of (lo, hi) bands (16-aligned, disjoint)."""
    import contextlib
    import concourse.bacc as bacc
    import concourse.tile as tile
    from concourse import mybir

    f32 = mybir.dt.float32
    wmax = max(sum(hi - lo for lo, hi in bands) for bands in windows)

    nc = bacc.Bacc("TRN2", target_bir_lowering=False, debug=False, num_devices=B)
    lhsT_d = nc.dram_tensor("lhsT", [4, N], f32, kind="ExternalInput")
    rhs_d = nc.dram_tensor("rhs", [4, N], f32, kind="ExternalInput")
    f_d = nc.dram_tensor("fvals", [1, N], f32, kind="ExternalInput")
    bias_d = nc.dram_tensor("biascol", [128, NTILES], f32, kind="ExternalInput")
    nf_d = nc.dram_tensor("nfcol", [128, NTILES], f32, kind="ExternalInput")
    out_d = nc.dram_tensor("partials", [128, NTILES], f32, kind="ExternalOutput")

    with tile.TileContext(nc) as tc:
        with (
            tc.tile_pool(name="const", bufs=1) as cpool,
            tc.tile_pool(name="work", bufs=3) as wpool,
            tc.tile_pool(name="small", bufs=3) as spool,
            tc.tile_pool(name="psum", bufs=2, space="PSUM") as ppool,
        ):
            lhsT_sb = cpool.tile([4, N], f32, tag="lhsT")
            rhs_sb = cpool.tile([4, N], f32, tag="rhs")
            f_row = cpool.tile([1, N], f32, tag="frow")
            F = cpool.tile([128, N], f32, tag="F")
            bias_sb = cpool.tile([128, NTILES], f32, tag="bias")
            nf_sb = cpool.tile([128, NTILES], f32, tag="nf")
            partials = cpool.tile([128, NTILES], f32, tag="partials")

            nc.sync.dma_start(lhsT_sb[:], lhsT_d.ap()[:])
            nc.sync.dma_start(rhs_sb[:], rhs_d.ap()[:])
            nc.sync.dma_start(f_row[:], f_d.ap()[:])
            nc.sync.dma_start(bias_sb[:], bias_d.ap()[:])
            nc.sync.dma_start(nf_sb[:], nf_d.ap()[:])
            nc.gpsimd.partition_broadcast(F[:], f_row[:])

            if rep > 1:
                kw = {}
                if hint:
                    kw["hint_engines"] = (
                        mybir.EngineType.DVE,
                        mybir.EngineType.Activation,
                        mybir.EngineType.PE,
                    )
                rep_ctx = tc.For_i(0, rep, 1, **kw)
            else:
                rep_ctx = contextlib.nullcontext()
            with rep_ctx:
                _emit_tiles(nc, mybir, windows, wmax, wpool, spool, ppool,
                            lhsT_sb, rhs_sb, F, bias_sb, nf_sb, partials)
            nc.sync.dma_start(out_d.ap()[:], partials[:])

    nc.compile()
    return nc


def _emit_tiles(nc, mybir, windows, wmax, wpool, spool, ppool,
                lhsT_sb, rhs_sb, F, bias_sb, nf_sb, partials):
    f32 = mybir.dt.float32
    for t in range(NTILES):
        bands = windows[t]
        w = sum(hi - lo for lo, hi in bands)
        assert w % NG == 0 and w >= 128, (t, w, bands)
        wg = w // NG
        # u0/G live in a "grouped" layout over the concatenated band columns:
        # concatenated element j sits at [g*wg + k] with j = k*NG + g, so
        # group g (a contiguous slice) holds every NG-th candidate.
        u0 = wpool.tile([128, wmax], f32, tag="u0")
        G = wpool.tile([128, wmax], f32, tag="G")
        u0g = u0[:, :w].rearrange("p (g k) -> p k g", g=NG)
        Gg = G[:, :w].rearrange("p (g k) -> p k g", g=NG)
        lhsT_t = lhsT_sb[:, 128 * t : 128 * (t + 1)]

        # per band: matmuls into a 512-aligned PSUM slice (a matmul may not
        # cross a PSUM bank boundary), then one ACT flush into u0's grouped
        # layout; G gets its own ACT from the F columns of the band.
        goff = 0
        psoff = PSUM_W  # force allocation on first band
        ps = None
        for lo, hi in bands:
            wb = hi - lo
            need = ((wb + 511) // 512) * 512
            if psoff + need > PSUM_W:
                ps = ppool.tile([128, PSUM_W], f32, tag="ps")
                psoff = 0
            for coff in range(0, wb, 512):
                cw = min(512, wb - coff)
                nc.tensor.matmul(
                    ps[:, psoff + coff : psoff + coff + cw],
                    lhsT_t,
                    rhs_sb[:, lo + coff : lo + coff + cw],
                    start=True,
                    stop=True,
                )
            nc.scalar.activation(
                u0g[:, goff // NG : (goff + wb) // NG, :],
                ps[:, psoff : psoff + wb].rearrange("p (k g) -> p k g", g=NG),
                mybir.ActivationFunctionType.Identity,
                bias=bias_sb[:, t : t + 1],
            )
            nc.scalar.activation(
                Gg[:, goff // NG : (goff + wb) // NG, :],
                F[:, lo:hi].rearrange("p (k g) -> p k g", g=NG),
                mybir.ActivationFunctionType.Square,
                bias=nf_sb[:, t : t + 1],
            )
            psoff += need
            goff += wb

        cand = spool.tile([128, 8 * NG], f32, tag="cand")
        for g in range(NG):
            nc.vector.max(
                out=cand[:, 8 * g : 8 * g + 8], in_=u0[:, g * wg : (g + 1) * wg]
            )
        m8a = spool.tile([128, 8], f32, tag="m8a")
        m8b = spool.tile([128, 8], f32, tag="m8b")
        m8c = spool.tile([128, 8], f32, tag="m8c")
        m8d = spool.tile([128, 8], f32, tag="m8d")
        v1 = spool.tile([128, 8 * NG], f32, tag="v1")
        v2 = spool.tile([128, 8 * NG], f32, tag="v2")
        v3 = spool.tile([128, 8 * NG], f32, tag="v3")
        nc.vector.max(out=m8a[:], in_=cand[:])
        nc.vector.match_replace(
            out=v1[:], in_to_replace=m8a[:], in_values=cand[:], imm_value=BIG_NEG
        )
        nc.vector.max(out=m8b[:], in_=v1[:])
        nc.vector.match_replace(
            out=v2[:], in_to_replace=m8b[:], in_values=v1[:], imm_value=BIG_NEG
        )
        nc.vector.max(out=m8c[:], in_=v2[:])
        nc.vector.match_replace(
            out=v3[:], in_to_replace=m8c[:], in_values=v2[:], imm_value=BIG_NEG
        )
        nc.vector.max(out=m8d[:], in_=v3[:])
        teff = spool.tile([128, 1], f32, tag="teff")
        nc.scalar.activation(
            teff[:], m8d[:, 7:8], mybir.ActivationFunctionType.Relu
        )
        scratch = wpool.tile([128, wmax], f32, tag="scratch")
        nc.vector.scalar_tensor_tensor(
            out=scratch[:, :w],
            in0=u0[:, :w],
            scalar=teff[:],
            in1=G[:, :w],
            op0=mybir.AluOpType.is_ge,
            op1=mybir.AluOpType.mult,
            accum_out=partials[:, t : t + 1],
        )


def _get_kernel(windows, rep=1, hint=False):
    key = (tuple(windows), rep, hint)
    if key not in _kernel_cache:
        _kernel_cache[key] = _build_bass(list(windows), rep=rep, hint=hint)
    return _kernel_cache[key]


def _prep_core(pos_b, f_b):
    """Preprocess one cloud -> (input map, per-(tile,bin) band dict)."""
    ox = np.argsort(pos_b[:, 0], kind="stable")
    px = pos_b[ox]
    # two-level order: x-bin (fixed rank edges), then y within the bin
    sub = np.concatenate(
        [
            BIN_EDGES[i]
            + np.argsort(px[BIN_EDGES[i] : BIN_EDGES[i + 1], 1], kind="stable")
            for i in range(NBINS)
        ]
    )
    order = ox[sub]
    p = pos_b[order].astype(np.float32)
    fs = f_b[order].astype(np.float32)
    c = (p.astype(np.float64) - 0.5)
    n = (c * c).sum(-1)
    c32 = c.astype(np.float32)

    lhsT = np.empty((4, N), np.float32)
    lhsT[0:3] = c32.T
    lhsT[3] = 1.0
    rhs = np.empty((4, N), np.float32)
    rhs[0:3] = 2.0 * c32.T
    rhs[3] = (-n).astype(np.float32)
    biascol = np.ascontiguousarray(
        (RSQ - n).astype(np.float32).reshape(NTILES, 128).T
    )
    nfcol = np.ascontiguousarray((-fs).reshape(NTILES, 128).T)
    fvals = fs.reshape(1, N)

    # exact per-(tile, bin) in-radius rank bands
    x64 = p[:, 0].astype(np.float64)
    y64 = p[:, 1].astype(np.float64)
    # x-range of each bin (in this cloud)
    bin_x = [
        (
            -np.inf if i == 0 else x64[BIN_EDGES[i] : BIN_EDGES[i + 1]].min(),
            np.inf if i == NBINS - 1 else x64[BIN_EDGES[i] : BIN_EDGES[i + 1]].max(),
        )
        for i in range(NBINS)
    ]
    bands = {}  # (t, bin) -> [lo, hi)
    for t in range(NTILES):
        xlo = x64[128 * t : 128 * (t + 1)].min() - RPAD
        xhi = x64[128 * t : 128 * (t + 1)].max() + RPAD
        ylo = y64[128 * t : 128 * (t + 1)].min() - RPAD
        yhi = y64[128 * t : 128 * (t + 1)].max() + RPAD
        for i in range(NBINS):
            blo, bhi = bin_x[i]
            if bhi < xlo or blo > xhi:
                continue
            e0, e1 = BIN_EDGES[i], BIN_EDGES[i + 1]
            lo = e0 + int(np.searchsorted(y64[e0:e1], ylo, side="left"))
            hi = e0 + int(np.searchsorted(y64[e0:e1], yhi, side="right"))
            if hi > lo:
                bands[(t, i)] = (lo, hi)
    in_map = {
        "lhsT": lhsT,
        "rhs": rhs,
        "fvals": fvals,
        "biascol": biascol,
        "nfcol": nfcol,
    }
    return in_map, bands


def prepare_inputs(pos, f):
    """Returns (in_maps, windows) for the 8 cores."""
    pos = np.asarray(pos, dtype=np.float32)
    f = np.asarray(f, dtype=np.float32)
    assert pos.shape == (B, N, 3), pos.shape
    assert f.shape == (B, N), f.shape
    in_maps = []
    union = {}
    for b in range(B):
        m, bands = _prep_core(pos[b], f[b])
        in_maps.append(m)
        for key, (lo, hi) in bands.items():
            if key in union:
                ulo, uhi = union[key]
                union[key] = (min(ulo, lo), max(uhi, hi))
            else:
                union[key] = (lo, hi)
    windows = []
    for t in range(NTILES):
        tb = []
        for i in range(NBINS):
            if (t, i) not in union:
                continue
            lo, hi = union[(t, i)]
            e0, e1 = BIN_EDGES[i], BIN_EDGES[i + 1]
            lo = max(e0, (lo // NG) * NG)
            hi = min(e1, ((hi + NG - 1) // NG) * NG)
            if hi > lo:
                tb.append((int(lo), int(hi)))
        windows.append(tuple(tb))
    return in_maps, windows


def finish(results):
    total = 0.0
    for rmap in results:
        total += rmap["partials"].astype(np.float64).sum()
    return np.asarray(0.5 * total / (B * N), dtype=np.float32)


def kernel(pos, f):
    from concourse.bass_utils import run_bass_kernel_spmd

    in_maps, windows = prepare_inputs(pos, f)
    nc = _get_kernel(windows)
    res = run_bass_kernel_spmd(nc, in_maps, list(range(B)))
    return finish(res.results)
